# Initial kernel scaffold
#
"""BertSelfAttention on 8 Trainium2 NeuronCores (Bass/Tile, SPMD).

Problem: B=2, S=2048, D=1024, H=16 heads, head_dim=64.
Sharding: core c handles batch b = c//4 and heads [4*(c%4), 4*(c%4)+4)
(data parallel on B x tensor parallel on heads). Scores stay core-local.

Per-core kernel (all matmuls fp32r = 11-mantissa-bit fp32, full PE rate):
  hsT [D,S] (host-pretransposed), W[:,256] slices.
  qT[hc, S] = Wq_s.T @ hsT    (head-col on partitions, +bq on DVE)
  kT stored zero-padded per head parity (kz_lo/kz_hi) so score matmuls
     contract over K=128 (avoids PE 64-row tiling mode switches).
  v [S, 4*65] with a per-head 65th column = exp(mask) ("ones" trick:
     folds both the softmax denominator and the additive mask).
  sT[k, q] = kT.T @ qT ; pT = exp(sT/8) (ACT, 1024-wide PSUM->SBUF)
  ctxT[65, q] += v_ext.T @ pT  (row 64 = softmax denominator)
  transpose ctxT -> [q, 65] (PE), divide by col 64 (DVE), DMA out.
Engine split: PE matmuls, ACT only exp, DVE all PSUM evacuation + div.

Math notes (exact transformations vs the reference):
  - bk dropped: scores[i,j] += q_i . bk is constant in j -> softmax invariant.
  - bv added host-side: softmax rows sum to 1 -> probs @ (1 x bv) = bv.
  - no max-subtraction: scores ~ N(0,1), exp range is tiny for fp32.
  - additive mask folded multiplicatively: exp(s+m) = exp(s)*exp(m).
"""

import numpy as np
from contextlib import ExitStack

B, S, D, H = 2, 2048, 1024, 16
HD = 64
N_CORES = 8
HPC = 4            # heads per core
CW = HPC * HD      # 256 output cols per core
KI = D // 128      # 8 contraction chunks
NQ = S // 512      # 4 q-chunks of 512
NSC = S // 128     # 16 s-chunks of 128

_NC_CACHE = []


def _build_nc():
    import concourse.bacc as bacc
    import concourse.mybir as mybir
    import concourse.tile as tile
    from concourse import masks

    F32 = mybir.dt.float32
    F32R = mybir.dt.float32r
    U32 = mybir.dt.uint32
    AF = mybir.ActivationFunctionType

    nc = bacc.Bacc("TRN2", target_bir_lowering=False, debug=False)

    hsT_d = nc.dram_tensor("hsT", [D, S], F32R, kind="ExternalInput")
    wq_d = nc.dram_tensor("wq", [D, CW], F32R, kind="ExternalInput")
    wk_d = nc.dram_tensor("wk", [D, CW], F32R, kind="ExternalInput")
    wv_d = nc.dram_tensor("wv", [D, CW], F32R, kind="ExternalInput")
    bq_d = nc.dram_tensor("bq", [CW], F32, kind="ExternalInput")
    mask_d = nc.dram_tensor("mask", [S], F32, kind="ExternalInput")
    out_d = nc.dram_tensor("out", [S, CW], F32, kind="ExternalOutput")

    hsT_r = hsT_d.rearrange("(ki p) s -> p ki s", p=128)
    wq_r = wq_d.rearrange("(ki p) n -> p ki n", p=128)
    wk_r = wk_d.rearrange("(ki p) n -> p ki n", p=128)
    wv_r = wv_d.rearrange("(ki p) n -> p ki n", p=128)
    bq_r = bq_d.rearrange("(j p) -> p j", p=128)
    mask_r = mask_d.rearrange("(sc p) -> p sc", p=128)

    with tile.TileContext(nc) as tc, ExitStack() as ctx:
        const = ctx.enter_context(tc.tile_pool(name="const", bufs=1))
        load = ctx.enter_context(tc.tile_pool(name="load", bufs=1))
        acts = ctx.enter_context(tc.tile_pool(name="acts", bufs=1))
        work = ctx.enter_context(tc.tile_pool(name="work", bufs=10))
        outp = ctx.enter_context(tc.tile_pool(name="outp", bufs=10))
        # PSUM: smm 2x[128,1024] = 4 banks, acc 4x[128,512-pad-1024?]: acc
        # tiles are [128,512] (1 bank) x4; transposes reuse the acc tag.
        smm = ctx.enter_context(tc.tile_pool(name="smm", bufs=2, space="PSUM"))
        ctxa = ctx.enter_context(tc.tile_pool(name="ctxa", bufs=4, space="PSUM"))

        # ---- constants ----
        ident = const.tile([128, 128], F32)
        masks.make_identity(nc, ident[:])
        bq_t = const.tile([128, 2], F32)
        nc.sync.dma_start(bq_t[:], bq_r)
        maskr = const.tile([128, 16], F32)
        nc.sync.dma_start(maskr[:], mask_r)
        emt = const.tile([128, 16], F32)
        nc.scalar.activation(emt[:], maskr[:], AF.Exp)

        # ---- loads: wq first, hsT by (n-chunk, ki) so QKV starts early ----
        hsT_t = load.tile([128, KI, S], F32R)
        wq_t = load.tile([128, KI, CW], F32R)
        wk_t = load.tile([128, KI, CW], F32R)
        wv_t = load.tile([128, KI, CW], F32R)
        # W on gpsimd (SWDGE queues) in parallel with hsT on sync (HWDGE);
        # hsT: n0 chunk first for early start, then one bulk DMA per ki.
        for ki in range(KI):
            nc.gpsimd.dma_start(wq_t[:, ki, :], wq_r[:, ki, :])
        for ki in range(KI):
            nc.gpsimd.dma_start(wk_t[:, ki, :], wk_r[:, ki, :])
        for ki in range(KI):
            nc.sync.dma_start(hsT_t[:, ki, 0:512], hsT_r[:, ki, 0:512])
        for ki in range(KI):
            nc.gpsimd.dma_start(wv_t[:, ki, :], wv_r[:, ki, :])
        for ki in range(KI):
            nc.sync.dma_start(hsT_t[:, ki, 512:2048], hsT_r[:, ki, 512:2048])

        # ---- persistent activations ----
        qT_t = acts.tile([128, 2, S], F32R)
        kz_lo = acts.tile([128, 2, S], F32R)
        kz_hi = acts.tile([128, 2, S], F32R)
        v_ext = acts.tile([128, NSC, HPC * (HD + 1)], F32R)
        nc.vector.memset(kz_lo[64:128, :, :].bitcast(U32), 0)
        nc.vector.memset(kz_hi[0:64, :, :].bitcast(U32), 0)

        # ---- Q^T and K^T (zero-padded) for one j-tile ----
        def emit_qk_n(j, n):
            if True:
                ps = smm.tile([128, 1024], F32, tag="mm", name=f"qk{j}_{n}")
                for ki in range(KI):
                    nc.tensor.matmul(
                        ps[:, 0:512], wq_t[:, ki, 128 * j:128 * (j + 1)],
                        hsT_t[:, ki, 512 * n:512 * (n + 1)],
                        start=(ki == 0), stop=(ki == KI - 1))
                for ki in range(KI):
                    nc.tensor.matmul(
                        ps[:, 512:1024], wk_t[:, ki, 128 * j:128 * (j + 1)],
                        hsT_t[:, ki, 512 * n:512 * (n + 1)],
                        start=(ki == 0), stop=(ki == KI - 1))
                nc.vector.tensor_scalar_add(qT_t[:, j, 512 * n:512 * (n + 1)],
                                            ps[:, 0:512], bq_t[:, j:j + 1])
                nc.vector.tensor_copy(kz_lo[0:64, j, 512 * n:512 * (n + 1)],
                                      ps[0:64, 512:1024])
                nc.vector.tensor_copy(kz_hi[64:128, j, 512 * n:512 * (n + 1)],
                                      ps[64:128, 512:1024])

        # ---- V with mask-scaled cols + exp(mask) column ----
        def emit_v():
            for sc in range(NSC):
                ps = smm.tile([128, 1024], F32, tag="mm", name=f"v{sc}")
                for ki in range(KI):
                    nc.tensor.matmul(
                        ps[:, 0:CW], hsT_t[:, ki, 128 * sc:128 * (sc + 1)],
                        wv_t[:, ki, :], start=(ki == 0), stop=(ki == KI - 1))
                for h in range(HPC):
                    nc.vector.tensor_scalar_mul(
                        v_ext[:, sc, 65 * h:65 * h + 64],
                        ps[:, 64 * h:64 * (h + 1)], emt[:, sc:sc + 1])
                    nc.vector.tensor_copy(
                        v_ext[:, sc, 65 * h + 64:65 * h + 65], emt[:, sc:sc + 1])

        def emit_scores(h, kc, psum_tag):
            j = h // 2
            kz = kz_lo if h % 2 == 0 else kz_hi
            pts = []
            for half in range(2):
                if psum_tag == "acc":
                    sp = ctxa.tile([128, 512], F32, tag="acc",
                                   name=f"pre{h}_{kc}_{half}")
                    spv = [sp[:], sp[:]]
                    off = [0, 0]
                    sp2 = ctxa.tile([128, 512], F32, tag="acc",
                                    name=f"pre2{h}_{kc}_{half}")
                    spv[1] = sp2[:]
                else:
                    sp = smm.tile([128, 1024], F32, tag="mm",
                                  name=f"sp{h}_{kc}_{half}")
                    spv = [sp[:, 0:512], sp[:, 512:1024]]
                for qh in range(2):
                    qc = 2 * half + qh
                    nc.tensor.matmul(
                        spv[qh],
                        kz[:, j, 128 * kc:128 * (kc + 1)],
                        qT_t[:, j, 512 * qc:512 * (qc + 1)],
                        start=True, stop=True)
                pth = work.tile([128, 1024], F32R, tag="pt",
                                name=f"pt{h}_{kc}_{half}")
                if psum_tag == "acc":
                    nc.scalar.activation(pth[:, 0:512], spv[0], AF.Exp,
                                         scale=0.125)
                    nc.scalar.activation(pth[:, 512:1024], spv[1], AF.Exp,
                                         scale=0.125)
                else:
                    nc.scalar.activation(pth[:], sp[:], AF.Exp, scale=0.125)
                pts.append(pth)
            return pts

        NPRE = 5
        for n in range(NQ):
            for j in range(2):
                emit_qk_n(j, n)
        # prefetch h0's first NPRE score/exp iterations into the v window,
        # using the otherwise-idle acc PSUM banks and idle ACT.
        pre0 = {kc: emit_scores(0, kc, "acc") for kc in range(NPRE)}
        emit_v()

        # ---- attention per head ----
        for h in range(HPC):
            acc = [ctxa.tile([128, 512], F32, tag="acc", name=f"acc{h}_{i}")
                   for i in range(NQ)]
            for kc in range(NSC):
                if h == 0 and kc in pre0:
                    pts = pre0[kc]
                else:
                    pts = emit_scores(h, kc, "mm")
                for qc in range(NQ):
                    nc.tensor.matmul(
                        acc[qc][0:65, :], v_ext[:, kc, 65 * h:65 * (h + 1)],
                        pts[qc // 2][:, 512 * (qc % 2):512 * (qc % 2 + 1)],
                        start=(kc == 0), stop=(kc == NSC - 1))
            for qc in range(NQ):
                cts = work.tile([65, 512], F32, tag="cts", bufs=4)
                nc.vector.tensor_copy(cts[:], acc[qc][0:65, :])
                for sq in range(4):
                    tp = ctxa.tile([128, 65], F32, tag="acc",
                                   name=f"tp{h}_{qc}_{sq}")
                    nc.tensor.transpose(tp[:, 0:65],
                                        cts[0:65, 128 * sq:128 * (sq + 1)],
                                        ident[0:65, 0:65])
                    rec = outp.tile([128, 1], F32, tag="rec")
                    nc.vector.reciprocal(rec[:], tp[:, 64:65])
                    ot = outp.tile([128, HD], F32, tag="ot")
                    nc.vector.tensor_scalar_mul(ot[:], tp[:, 0:HD], rec[:, 0:1])
                    row = 512 * qc + 128 * sq
                    nc.sync.dma_start(
                        out_d[row:row + 128, HD * h:HD * (h + 1)], ot[:])

    nc.finalize()
    return nc


def _get_nc():
    if not _NC_CACHE:
        _NC_CACHE.append(_build_nc())
    return _NC_CACHE[0]


def _shard_inputs(hidden_states, attention_mask, Wq, bq, Wk, Wv):
    hsT = [np.ascontiguousarray(hidden_states[b].T) for b in range(B)]
    in_maps = []
    for c in range(N_CORES):
        b, g = divmod(c, N_CORES // B)
        cs = slice(CW * g, CW * (g + 1))
        in_maps.append({
            "hsT": hsT[b],
            "wq": np.ascontiguousarray(Wq[:, cs]),
            "wk": np.ascontiguousarray(Wk[:, cs]),
            "wv": np.ascontiguousarray(Wv[:, cs]),
            "bq": np.ascontiguousarray(bq[cs]),
            "mask": np.ascontiguousarray(attention_mask[b, 0, 0, :]),
        })
    return in_maps


def kernel(hidden_states, attention_mask, Wq, bq, Wk, bk, Wv, bv):
    from concourse.bass_utils import run_bass_kernel_spmd

    hidden_states = np.asarray(hidden_states, dtype=np.float32)
    attention_mask = np.asarray(attention_mask, dtype=np.float32)
    Wq = np.asarray(Wq, dtype=np.float32)
    Wk = np.asarray(Wk, dtype=np.float32)
    Wv = np.asarray(Wv, dtype=np.float32)
    bq = np.asarray(bq, dtype=np.float32)
    bv = np.asarray(bv, dtype=np.float32)

    in_maps = _shard_inputs(hidden_states, attention_mask, Wq, bq, Wk, Wv)
    res = run_bass_kernel_spmd(_get_nc(), in_maps, core_ids=list(range(N_CORES)))

    out = np.empty((B, S, D), dtype=np.float32)
    for c in range(N_CORES):
        b, g = divmod(c, N_CORES // B)
        out[b, :, CW * g:CW * (g + 1)] = res.results[c]["out"]
    out += bv  # exact: probs rows sum to 1
    return out



# revision 1
# speedup vs baseline: 1.1437x; 1.1437x over previous
"""BertSelfAttention on 8 Trainium2 NeuronCores (Bass/Tile, SPMD).

Problem: B=2, S=2048, D=1024, H=16 heads, head_dim=64.
Sharding: core c handles batch b = c//4 and heads [4*(c%4), 4*(c%4)+4)
(data parallel on B x tensor parallel on heads). Scores stay core-local.

Per-core kernel (all matmuls fp32r = 11-mantissa-bit fp32, full PE rate):
  hsT [D,S] (host-pretransposed), W[:,256] slices.
  qT[hc, S] = Wq_s.T @ hsT    (head-col on partitions, +bq on DVE)
  kT stored zero-padded per head parity (kz_lo/kz_hi) so score matmuls
     contract over K=128 (avoids PE 64-row tiling mode switches).
  v [S, 4*65] with a per-head 65th column = exp(mask) ("ones" trick:
     folds both the softmax denominator and the additive mask).
  sT[k, q] = kT.T @ qT ; pT = exp(sT/8) (ACT, 1024-wide PSUM->SBUF)
  ctxT[65, q] += v_ext.T @ pT  (row 64 = softmax denominator)
  transpose ctxT -> [q, 65] (PE), divide by col 64 (DVE), DMA out.
Engine split: PE matmuls, ACT only exp, DVE all PSUM evacuation + div.

Math notes (exact transformations vs the reference):
  - bk dropped: scores[i,j] += q_i . bk is constant in j -> softmax invariant.
  - bv added host-side: softmax rows sum to 1 -> probs @ (1 x bv) = bv.
  - no max-subtraction: scores ~ N(0,1), exp range is tiny for fp32.
  - additive mask folded multiplicatively: exp(s+m) = exp(s)*exp(m).
"""

import numpy as np
from contextlib import ExitStack

B, S, D, H = 2, 2048, 1024, 16
HD = 64
N_CORES = 8
HPC = 4            # heads per core
CW = HPC * HD      # 256 output cols per core
KI = D // 128      # 8 contraction chunks
NQ = S // 512      # 4 q-chunks of 512
NSC = S // 128     # 16 s-chunks of 128

_NC_CACHE = []


def _build_nc():
    import concourse.bacc as bacc
    import concourse.mybir as mybir
    import concourse.tile as tile
    from concourse import masks

    F32 = mybir.dt.float32
    F32R = mybir.dt.float32r
    U32 = mybir.dt.uint32
    AF = mybir.ActivationFunctionType

    nc = bacc.Bacc("TRN2", target_bir_lowering=False, debug=False)

    hsT_d = nc.dram_tensor("hsT", [D, S], F32R, kind="ExternalInput")
    wq_d = nc.dram_tensor("wq", [D, CW], F32R, kind="ExternalInput")
    wk_d = nc.dram_tensor("wk", [D, CW], F32R, kind="ExternalInput")
    wv_d = nc.dram_tensor("wv", [D, CW], F32R, kind="ExternalInput")
    bq_d = nc.dram_tensor("bq", [CW], F32, kind="ExternalInput")
    mask_d = nc.dram_tensor("mask", [S], F32, kind="ExternalInput")
    out_d = nc.dram_tensor("out", [S, CW], F32, kind="ExternalOutput")

    hsT_r = hsT_d.rearrange("(ki p) s -> p ki s", p=128)
    wq_r = wq_d.rearrange("(ki p) n -> p ki n", p=128)
    wk_r = wk_d.rearrange("(ki p) n -> p ki n", p=128)
    wv_r = wv_d.rearrange("(ki p) n -> p ki n", p=128)
    bq_r = bq_d.rearrange("(j p) -> p j", p=128)
    mask_r = mask_d.rearrange("(sc p) -> p sc", p=128)

    with tile.TileContext(nc) as tc, ExitStack() as ctx:
        const = ctx.enter_context(tc.tile_pool(name="const", bufs=1))
        load = ctx.enter_context(tc.tile_pool(name="load", bufs=1))
        acts = ctx.enter_context(tc.tile_pool(name="acts", bufs=1))
        work = ctx.enter_context(tc.tile_pool(name="work", bufs=10))
        outp = ctx.enter_context(tc.tile_pool(name="outp", bufs=10))
        # PSUM: smm 2x[128,1024] = 4 banks, acc 4x[128,512-pad-1024?]: acc
        # tiles are [128,512] (1 bank) x4; transposes reuse the acc tag.
        smm = ctx.enter_context(tc.tile_pool(name="smm", bufs=2, space="PSUM"))
        ctxa = ctx.enter_context(tc.tile_pool(name="ctxa", bufs=4, space="PSUM"))

        # ---- constants ----
        ident = const.tile([128, 128], F32)
        masks.make_identity(nc, ident[:])
        bq_t = const.tile([128, 2], F32)
        nc.sync.dma_start(bq_t[:], bq_r)
        maskr = const.tile([128, 16], F32)
        nc.sync.dma_start(maskr[:], mask_r)
        emt = const.tile([128, 16], F32)
        nc.scalar.activation(emt[:], maskr[:], AF.Exp)

        # ---- loads: wq first, hsT by (n-chunk, ki) so QKV starts early ----
        hsT_t = load.tile([128, KI, S], F32R)
        wq_t = load.tile([128, KI, CW], F32R)
        wk_t = load.tile([128, KI, CW], F32R)
        wv_t = load.tile([128, KI, CW], F32R)
        # W on gpsimd (SWDGE queues) in parallel with hsT on sync (HWDGE);
        # hsT: n0 chunk first for early start, then one bulk DMA per ki.
        for ki in range(KI):
            nc.gpsimd.dma_start(wq_t[:, ki, :], wq_r[:, ki, :])
        for ki in range(KI):
            nc.gpsimd.dma_start(wk_t[:, ki, :], wk_r[:, ki, :])
        for ki in range(KI):
            nc.sync.dma_start(hsT_t[:, ki, 0:512], hsT_r[:, ki, 0:512])
        for ki in range(KI):
            nc.gpsimd.dma_start(wv_t[:, ki, :], wv_r[:, ki, :])
        for ki in range(KI):
            nc.sync.dma_start(hsT_t[:, ki, 512:2048], hsT_r[:, ki, 512:2048])

        # ---- persistent activations ----
        qT_t = acts.tile([128, 2, S], F32R)
        kz_lo = acts.tile([128, 2, S], F32R)
        kz_hi = acts.tile([128, 2, S], F32R)
        v_ext = acts.tile([128, NSC, HPC * (HD + 1)], F32R)
        nc.vector.memset(kz_lo[64:128, :, :].bitcast(U32), 0)
        nc.vector.memset(kz_hi[0:64, :, :].bitcast(U32), 0)

        # ---- Q^T and K^T (zero-padded) for one j-tile ----
        def emit_qk_n(j, n):
            if True:
                ps = smm.tile([128, 1024], F32, tag="mm", name=f"qk{j}_{n}")
                for ki in range(KI):
                    nc.tensor.matmul(
                        ps[:, 0:512], wq_t[:, ki, 128 * j:128 * (j + 1)],
                        hsT_t[:, ki, 512 * n:512 * (n + 1)],
                        start=(ki == 0), stop=(ki == KI - 1))
                for ki in range(KI):
                    nc.tensor.matmul(
                        ps[:, 512:1024], wk_t[:, ki, 128 * j:128 * (j + 1)],
                        hsT_t[:, ki, 512 * n:512 * (n + 1)],
                        start=(ki == 0), stop=(ki == KI - 1))
                nc.vector.tensor_scalar_add(qT_t[:, j, 512 * n:512 * (n + 1)],
                                            ps[:, 0:512], bq_t[:, j:j + 1])
                nc.vector.tensor_copy(kz_lo[0:64, j, 512 * n:512 * (n + 1)],
                                      ps[0:64, 512:1024])
                nc.vector.tensor_copy(kz_hi[64:128, j, 512 * n:512 * (n + 1)],
                                      ps[64:128, 512:1024])

        # ---- V with mask-scaled cols + exp(mask) column ----
        def emit_v():
            for sc in range(NSC):
                ps = smm.tile([128, 1024], F32, tag="mm", name=f"v{sc}")
                for ki in range(KI):
                    nc.tensor.matmul(
                        ps[:, 0:CW], hsT_t[:, ki, 128 * sc:128 * (sc + 1)],
                        wv_t[:, ki, :], start=(ki == 0), stop=(ki == KI - 1))
                for h in range(HPC):
                    nc.vector.tensor_scalar_mul(
                        v_ext[:, sc, 65 * h:65 * h + 64],
                        ps[:, 64 * h:64 * (h + 1)], emt[:, sc:sc + 1])
                    nc.vector.tensor_copy(
                        v_ext[:, sc, 65 * h + 64:65 * h + 65], emt[:, sc:sc + 1])

        def emit_scores(h, kc, psum_tag):
            j = h // 2
            kz = kz_lo if h % 2 == 0 else kz_hi
            pts = []
            for half in range(2):
                if psum_tag == "acc":
                    sp = ctxa.tile([128, 512], F32, tag="acc",
                                   name=f"pre{h}_{kc}_{half}")
                    spv = [sp[:], sp[:]]
                    off = [0, 0]
                    sp2 = ctxa.tile([128, 512], F32, tag="acc",
                                    name=f"pre2{h}_{kc}_{half}")
                    spv[1] = sp2[:]
                else:
                    sp = smm.tile([128, 1024], F32, tag="mm",
                                  name=f"sp{h}_{kc}_{half}")
                    spv = [sp[:, 0:512], sp[:, 512:1024]]
                for qh in range(2):
                    qc = 2 * half + qh
                    nc.tensor.matmul(
                        spv[qh],
                        kz[:, j, 128 * kc:128 * (kc + 1)],
                        qT_t[:, j, 512 * qc:512 * (qc + 1)],
                        start=True, stop=True)
                pth = work.tile([128, 1024], F32R, tag="pt",
                                name=f"pt{h}_{kc}_{half}")
                if psum_tag == "acc":
                    nc.scalar.activation(pth[:, 0:512], spv[0], AF.Exp,
                                         scale=0.125)
                    nc.scalar.activation(pth[:, 512:1024], spv[1], AF.Exp,
                                         scale=0.125)
                else:
                    nc.scalar.activation(pth[:], sp[:], AF.Exp, scale=0.125)
                pts.append(pth)
            return pts

        NPRE = 5
        for n in range(NQ):
            for j in range(2):
                emit_qk_n(j, n)
        # prefetch h0's first NPRE score/exp iterations into the v window,
        # using the otherwise-idle acc PSUM banks and idle ACT.
        pre0 = {kc: emit_scores(0, kc, "acc") for kc in range(NPRE)}
        emit_v()

        # ---- attention per head ----
        for h in range(HPC):
            acc = [ctxa.tile([128, 512], F32, tag="acc", name=f"acc{h}_{i}")
                   for i in range(NQ)]
            for kc in range(NSC):
                if h == 0 and kc in pre0:
                    pts = pre0[kc]
                else:
                    pts = emit_scores(h, kc, "mm")
                for qc in range(NQ):
                    nc.tensor.matmul(
                        acc[qc][0:65, :], v_ext[:, kc, 65 * h:65 * (h + 1)],
                        pts[qc // 2][:, 512 * (qc % 2):512 * (qc % 2 + 1)],
                        start=(kc == 0), stop=(kc == NSC - 1))
            for qc in range(NQ):
                cts = work.tile([65, 512], F32, tag="cts", bufs=4)
                nc.vector.tensor_copy(cts[:], acc[qc][0:65, :])
                for sq in range(4):
                    tp = ctxa.tile([128, 65], F32, tag="acc",
                                   name=f"tp{h}_{qc}_{sq}")
                    nc.tensor.transpose(tp[:, 0:65],
                                        cts[0:65, 128 * sq:128 * (sq + 1)],
                                        ident[0:65, 0:65])
                    rec = outp.tile([128, 1], F32, tag="rec")
                    nc.vector.reciprocal(rec[:], tp[:, 64:65])
                    ot = outp.tile([128, HD], F32, tag="ot")
                    nc.vector.tensor_scalar_mul(ot[:], tp[:, 0:HD], rec[:, 0:1])
                    row = 512 * qc + 128 * sq
                    nc.sync.dma_start(
                        out_d[row:row + 128, HD * h:HD * (h + 1)], ot[:])

    nc.finalize()
    return nc


def _get_nc():
    if not _NC_CACHE:
        _NC_CACHE.append(_build_nc())
    return _NC_CACHE[0]


def _shard_inputs(hidden_states, attention_mask, Wq, bq, Wk, Wv):
    hsT = [np.ascontiguousarray(hidden_states[b].T) for b in range(B)]
    in_maps = []
    for c in range(N_CORES):
        b, g = divmod(c, N_CORES // B)
        cs = slice(CW * g, CW * (g + 1))
        in_maps.append({
            "hsT": hsT[b],
            "wq": np.ascontiguousarray(Wq[:, cs]),
            "wk": np.ascontiguousarray(Wk[:, cs]),
            "wv": np.ascontiguousarray(Wv[:, cs]),
            "bq": np.ascontiguousarray(bq[cs]),
            "mask": np.ascontiguousarray(attention_mask[b, 0, 0, :]),
        })
    return in_maps


def kernel(hidden_states, attention_mask, Wq, bq, Wk, bk, Wv, bv):
    from concourse.bass_utils import run_bass_kernel_spmd

    hidden_states = np.asarray(hidden_states, dtype=np.float32)
    attention_mask = np.asarray(attention_mask, dtype=np.float32)
    Wq = np.asarray(Wq, dtype=np.float32)
    Wk = np.asarray(Wk, dtype=np.float32)
    Wv = np.asarray(Wv, dtype=np.float32)
    bq = np.asarray(bq, dtype=np.float32)
    bv = np.asarray(bv, dtype=np.float32)

    in_maps = _shard_inputs(hidden_states, attention_mask, Wq, bq, Wk, Wv)
    res = run_bass_kernel_spmd(_get_nc(), in_maps, core_ids=list(range(N_CORES)))

    out = np.empty((B, S, D), dtype=np.float32)
    for c in range(N_CORES):
        b, g = divmod(c, N_CORES // B)
        out[b, :, CW * g:CW * (g + 1)] = res.results[c]["out"]
    out += bv  # exact: probs rows sum to 1
    return out

